# revision 4
# baseline (speedup 1.0000x reference)
"""Trainium2 Bass kernel for nn_AttentionPattern_83820581749443.

Single-head attention, B=4, S=2048, D=1024, fp32 I/O:
    Q = x@Wq.T+bq; K = x@Wk.T+bk; V = x@Wv.T+bv
    scores = (Q@K.T)/sqrt(D) * gauss_mask(key_pos)
    out = softmax(scores) @ V;  y = out@Wo.T+bo

Sharding: 8 cores, core c handles batch b=c//2, query rows q0=(c%2)*1024
... q0+1024. Each core computes K/V for its full batch (redundantly with
its pair core) — fully data-parallel, no collectives.

Per-core kernel (all matmul operands bf16, fp32 PSUM accumulation):
  - x (rolled so queries are rows 0:1024) is cast to bf16, bounced via
    DRAM and DMA-transposed into xT [d, m] layout.
  - Weights likewise -> WT [d, n].
  - Q.T[df, q] / K.T[df, k] projections:  lhsT=WT chunk, rhs=xT.
  - V[k, dv] natural:                     lhsT=xT chunk, rhs=WvT.
  - scores.T[k, q] per (k-chunk, q-chunk): lhsT=KT chunk, rhs=QT chunk.
  - P = exp(scores.T * mask[k]/sqrt(D)) via ACT with per-partition scale
    (no max subtraction: |z| <= ~8 so exp is safe in fp32).
  - out.T[dv, q] accumulated over k-chunks: lhsT=V chunk, rhs=P.
  - denom[q] via ones-matmul over P; transposed to a per-partition column
    with a small SWDGE scatter DMA; reciprocal on DVE.
  - y[q, n] = (outT.T @ WoT) * recip[q] + bo, streamed to DRAM.
"""

import os
import numpy as np
import ml_dtypes

import concourse.bass as bass
import concourse.bacc as bacc
import concourse.mybir as mybir
import concourse.tile as tile
from concourse.bass_utils import run_bass_kernel_spmd

P = 128
B, S, D = 4, 2048, 1024
NCORES = 8
QL = S * B // NCORES          # 1024 queries per core
DT = mybir.dt

LAST_EXEC_TIME_NS = None
_CACHE = {}


def _build():
    nc = bacc.Bacc("TRN2", target_bir_lowering=False, debug=False,
                   enable_asserts=True, num_devices=NCORES)

    x_in = nc.dram_tensor("x", [S, D], DT.float32, kind="ExternalInput")
    w_in = {w: nc.dram_tensor(w, [D, D], DT.float32, kind="ExternalInput")
            for w in ("Wq", "Wk", "Wv", "Wo")}
    mask_in = nc.dram_tensor("mask2d", [P, S // P], DT.float32,
                             kind="ExternalInput")
    bq_in = nc.dram_tensor("bq2d", [P, D // P], DT.float32, kind="ExternalInput")
    bk_in = nc.dram_tensor("bk2d", [P, D // P], DT.float32, kind="ExternalInput")
    bv_in = nc.dram_tensor("bv2d", [1, D], DT.float32, kind="ExternalInput")
    bo_in = nc.dram_tensor("bo2d", [1, D], DT.float32, kind="ExternalInput")
    y_out = nc.dram_tensor("y", [QL, D], DT.float32, kind="ExternalOutput")

    DC = D // P       # 8 d-chunks
    KC = S // P       # 16 k-chunks
    QCH = 256         # query chunk (psum-bank limited)
    NQC = QL // QCH   # 4 query chunks

    with tile.TileContext(nc) as tc:
        with (
            tc.tile_pool(name="const", bufs=1) as cpool,
            tc.tile_pool(name="big", bufs=1) as big,
            tc.tile_pool(name="wpool", bufs=2) as wpool,
            tc.tile_pool(name="stage", bufs=3) as stage,
            tc.tile_pool(name="ppool", bufs=4) as ppool,
            tc.tile_pool(name="otpool", bufs=2) as otpool,
            tc.tile_pool(name="ypool", bufs=3) as ypool,
            tc.tile_pool(name="small", bufs=2) as small,
            tc.tile_pool(name="psmm", bufs=3, space="PSUM") as psmm,
            tc.tile_pool(name="psout", bufs=4, space="PSUM") as psout,
            tc.tile_pool(name="psden", bufs=1, space="PSUM") as psden,
            tc.tile_pool(name="dram", bufs=1, space="DRAM") as dram,
        ):
            # ---- constants ----
            mask_sb = cpool.tile([P, KC], DT.float32, tag="mask")
            nc.sync.dma_start(mask_sb[:], mask_in[:])
            bq_sb = cpool.tile([P, DC], DT.float32, tag="bq")
            nc.sync.dma_start(bq_sb[:], bq_in[:])
            bk_sb = cpool.tile([P, DC], DT.float32, tag="bk")
            nc.sync.dma_start(bk_sb[:], bk_in[:])
            bv_bc = cpool.tile([P, D], DT.float32, tag="bv")
            nc.sync.dma_start(bv_bc[:], bv_in[:].to_broadcast((P, D)))
            bo_bc = cpool.tile([P, D], DT.float32, tag="bo")
            nc.sync.dma_start(bo_bc[:], bo_in[:].to_broadcast((P, D)))
            ones = cpool.tile([P, 1], DT.bfloat16, tag="ones")
            nc.vector.memset(ones[:], 1.0)

            # ---- cast x to bf16 via DRAM bounce, DMA-transpose to xT ----
            xbf = dram.tile([S, D], DT.bfloat16, tag="xbf")
            for mi in range(S // P):
                xs = stage.tile([P, D], DT.float32, tag="xs32")
                nc.sync.dma_start(xs[:], x_in[mi * P:(mi + 1) * P, :])
                xc = stage.tile([P, D], DT.bfloat16, tag="xs16")
                nc.vector.tensor_copy(xc[:], xs[:])
                nc.sync.dma_start(xbf[mi * P:(mi + 1) * P, :], xc[:])
            xt = big.tile([P, DC, S], DT.bfloat16, tag="xt")
            for dj in range(DC):
                for mb in range(4):
                    nc.sync.dma_start_transpose(
                        xt[:, dj, mb * 512:(mb + 1) * 512],
                        xbf[mb * 512:(mb + 1) * 512, dj * P:(dj + 1) * P],
                    )

            def prep_weight(name):
                wbf = dram.tile([D, D], DT.bfloat16, tag="wbf")
                for mi in range(DC):
                    ws = stage.tile([P, D], DT.float32, tag="xs32")
                    nc.sync.dma_start(ws[:], w_in[name][mi * P:(mi + 1) * P, :])
                    wc = stage.tile([P, D], DT.bfloat16, tag="xs16")
                    nc.vector.tensor_copy(wc[:], ws[:])
                    nc.sync.dma_start(wbf[mi * P:(mi + 1) * P, :], wc[:])
                wt = wpool.tile([P, DC, D], DT.bfloat16, tag="wT")
                for dj in range(DC):
                    for mb in range(2):
                        nc.sync.dma_start_transpose(
                            wt[:, dj, mb * 512:(mb + 1) * 512],
                            wbf[mb * 512:(mb + 1) * 512, dj * P:(dj + 1) * P],
                        )
                return wt

            # ---- Q.T projection: [df, q] over q rows 0:QL ----
            wqt = prep_weight("Wq")
            qt = big.tile([P, DC, QL], DT.bfloat16, tag="qt")
            for nq in range(DC):
                for qh in range(QL // 512):
                    ps = psmm.tile([P, 512], DT.float32, tag="mm")
                    for dc in range(DC):
                        nc.tensor.matmul(
                            ps[:], wqt[:, dc, nq * P:(nq + 1) * P],
                            xt[:, dc, qh * 512:(qh + 1) * 512],
                            start=(dc == 0), stop=(dc == DC - 1))
                    nc.vector.tensor_scalar_add(
                        qt[:, nq, qh * 512:(qh + 1) * 512], ps[:],
                        bq_sb[:, nq:nq + 1])

            # ---- K.T projection: [df, k] over all S keys ----
            wkt = prep_weight("Wk")
            kt = big.tile([P, DC, S], DT.bfloat16, tag="kt")
            for nk in range(DC):
                for kh in range(S // 512):
                    ps = psmm.tile([P, 512], DT.float32, tag="mm")
                    for dc in range(DC):
                        nc.tensor.matmul(
                            ps[:], wkt[:, dc, nk * P:(nk + 1) * P],
                            xt[:, dc, kh * 512:(kh + 1) * 512],
                            start=(dc == 0), stop=(dc == DC - 1))
                    nc.vector.tensor_scalar_add(
                        kt[:, nk, kh * 512:(kh + 1) * 512], ps[:],
                        bk_sb[:, nk:nk + 1])

            # ---- V projection (natural layout): [k, dv] ----
            wvt = prep_weight("Wv")
            v = big.tile([P, KC, D], DT.bfloat16, tag="v")
            for kc in range(KC):
                for dh in range(D // 512):
                    ps = psmm.tile([P, 512], DT.float32, tag="mm")
                    for dc in range(DC):
                        nc.tensor.matmul(
                            ps[:], xt[:, dc, kc * P:(kc + 1) * P],
                            wvt[:, dc, dh * 512:(dh + 1) * 512],
                            start=(dc == 0), stop=(dc == DC - 1))
                    nc.vector.tensor_tensor(
                        v[:, kc, dh * 512:(dh + 1) * 512], ps[:],
                        bv_bc[:, dh * 512:(dh + 1) * 512],
                        mybir.AluOpType.add)

            wot = prep_weight("Wo")

            # ---- attention + output projection, per query chunk ----
            for qc in range(NQC):
                q0 = qc * QCH
                out_ps = [psout.tile([P, 2, QCH], DT.float32, tag="outps",
                                     name=f"outps_{qc}_{j}")
                          for j in range(4)]
                den_ps = psden.tile([1, QCH], DT.float32, tag="den")

                def scores_chunk(kc):
                    s_ps = psmm.tile([P, QCH], DT.float32, tag="mm")
                    for dc in range(DC):
                        nc.tensor.matmul(
                            s_ps[:], kt[:, dc, kc * P:(kc + 1) * P],
                            qt[:, dc, q0:q0 + QCH],
                            start=(dc == 0), stop=(dc == DC - 1))
                    p_t = ppool.tile([P, QCH], DT.bfloat16, tag="p")
                    nc.scalar.activation(p_t[:], s_ps[:],
                                         mybir.ActivationFunctionType.Exp,
                                         scale=mask_sb[:, kc:kc + 1])
                    return p_t

                # software-pipelined: scores(kc+1) issued before V-matmuls(kc)
                p_prev = scores_chunk(0)
                for kc in range(KC):
                    p_next = scores_chunk(kc + 1) if kc + 1 < KC else None
                    first, last = (kc == 0), (kc == KC - 1)
                    for dvc in range(DC):
                        # start=True clears has_written for the WHOLE bank, so
                        # only the first write of each bank-sharing pair may
                        # set it; the second overwrites via cleared bits.
                        nc.tensor.matmul(
                            out_ps[dvc // 2][:, dvc % 2, :],
                            v[:, kc, dvc * P:(dvc + 1) * P], p_prev[:],
                            start=(first and dvc % 2 == 0), stop=last)
                    nc.tensor.matmul(den_ps[:], ones[:], p_prev[:],
                                     start=first, stop=last)
                    p_prev = p_next

                # denominator -> per-partition reciprocal column [128, 2]
                # (cross-partition transpose of a [1, 256] row via DRAM bounce)
                dsb = small.tile([1, QCH], DT.float32, tag="dsb")
                nc.vector.tensor_copy(dsb[:], den_ps[:])
                dtmp = dram.tile([QCH], DT.float32, tag="dtmp",
                                 name=f"dtmp_{qc}")
                nc.sync.dma_start(dtmp.rearrange("(a q) -> a q", a=1), dsb[:])
                dcol = small.tile([P, QCH // P], DT.float32, tag="dcol")
                nc.sync.dma_start(dcol[:],
                                  dtmp.rearrange("(j p) -> p j", p=P))
                rcol = small.tile([P, QCH // P], DT.float32, tag="rcol")
                nc.vector.reciprocal(rcol[:], dcol[:])

                # outT psum -> sbuf bf16 (unnormalized)
                ot = otpool.tile([P, DC, QCH], DT.bfloat16, tag="ot")
                for j in range(4):
                    nc.vector.tensor_copy(ot[:, 2 * j:2 * j + 2, :],
                                          out_ps[j][:])

                # y[q, n] = (ot.T @ WoT) * recip[q] + bo
                for qs in range(QCH // P):
                    for nh in range(D // 512):
                        y_ps = psmm.tile([P, 512], DT.float32, tag="mm")
                        for dvc in range(DC):
                            nc.tensor.matmul(
                                y_ps[:], ot[:, dvc, qs * P:(qs + 1) * P],
                                wot[:, dvc, nh * 512:(nh + 1) * 512],
                                start=(dvc == 0), stop=(dvc == DC - 1))
                        ysb = ypool.tile([P, 512], DT.float32, tag="y")
                        nc.vector.tensor_scalar_mul(ysb[:], y_ps[:],
                                                    rcol[:, qs:qs + 1])
                        nc.vector.tensor_tensor(
                            ysb[:], ysb[:], bo_bc[:, nh * 512:(nh + 1) * 512],
                            mybir.AluOpType.add)
                        nc.sync.dma_start(
                            y_out[q0 + qs * P:q0 + (qs + 1) * P,
                                  nh * 512:(nh + 1) * 512], ysb[:])

    nc.compile()
    return nc


def _host_inputs(x, Wq, bq, Wk, bk, Wv, bv, Wo, bo):
    pos = np.arange(S, dtype=np.float32)
    gauss = np.exp((-0.5 * ((pos - S / 2) / (S / 4)) ** 2).astype(np.float32))
    scale_vec = (gauss / np.float32(np.sqrt(np.float32(D)))).astype(np.float32)

    common = {
        "Wq": np.ascontiguousarray(Wq, np.float32),
        "Wk": np.ascontiguousarray(Wk, np.float32),
        "Wv": np.ascontiguousarray(Wv, np.float32),
        "Wo": np.ascontiguousarray(Wo, np.float32),
        "bq2d": np.ascontiguousarray(bq.reshape(D // P, P).T, np.float32),
        "bk2d": np.ascontiguousarray(bk.reshape(D // P, P).T, np.float32),
        "bv2d": np.ascontiguousarray(bv.reshape(1, D), np.float32),
        "bo2d": np.ascontiguousarray(bo.reshape(1, D), np.float32),
    }
    in_maps = []
    for c in range(NCORES):
        b, h = divmod(c, 2)
        q0 = h * QL
        x_roll = np.roll(np.asarray(x[b], np.float32), -q0, axis=0)
        m_roll = np.roll(scale_vec, -q0)
        in_maps.append(dict(
            common,
            x=np.ascontiguousarray(x_roll),
            mask2d=np.ascontiguousarray(m_roll.reshape(S // P, P).T,
                                        np.float32),
        ))
    return in_maps


def kernel(x, Wq, bq, Wk, bk, Wv, bv, Wo, bo):
    global LAST_EXEC_TIME_NS
    x = np.asarray(x, np.float32)
    if "nc" not in _CACHE:
        _CACHE["nc"] = _build()
    nc = _CACHE["nc"]
    in_maps = _host_inputs(x, np.asarray(Wq), np.asarray(bq), np.asarray(Wk),
                           np.asarray(bk), np.asarray(Wv), np.asarray(bv),
                           np.asarray(Wo), np.asarray(bo))
    trace = bool(int(os.environ.get("BASS_KERNEL_TRACE", "0")))
    res = run_bass_kernel_spmd(nc, in_maps, core_ids=list(range(NCORES)),
                               trace=trace)
    LAST_EXEC_TIME_NS = res.exec_time_ns
    y = np.empty((B, S, D), np.float32)
    for c in range(NCORES):
        b, h = divmod(c, 2)
        y[b, h * QL:(h + 1) * QL] = res.results[c]["y"]
    return y


# revision 5
# speedup vs baseline: 1.2337x; 1.2337x over previous
"""Trainium2 Bass kernel for nn_AttentionPattern_83820581749443.

Single-head attention, B=4, S=2048, D=1024, fp32 I/O:
    Q = x@Wq.T+bq; K = x@Wk.T+bk; V = x@Wv.T+bv
    scores = (Q@K.T)/sqrt(D) * gauss_mask(key_pos)
    out = softmax(scores) @ V;  y = out@Wo.T+bo

Sharding: 8 cores, core c handles batch b=c//2, query rows q0=(c%2)*1024
... q0+1024. Each core computes K/V for its full batch (redundantly with
its pair core) — fully data-parallel, no collectives. Inputs are rolled
host-side so each core's queries are rows 0:1024 (attention over keys is
permutation-invariant; the gaussian mask is rolled to match).

Per-core kernel (matmul operands bf16, fp32 PSUM accumulation):
  - x cast to bf16 by SWDGE casting DMA (DRAM->DRAM), DMA-transposed into
    xT [d, m]; weights likewise -> WT [d, n].
  - Q.T[df, q] / K.T[df, k]: lhsT=WT chunk, rhs=xT.   V[k, dv]: lhsT=xT.
  - scores.T[k, q]: lhsT=KT chunk, rhs=QT chunk.
  - P = exp(scores.T * mask[k]/sqrt(D)) on ACT with per-partition scale
    (no max subtraction needed: |z| <= ~8).
  - out.T[dv, q] += V-chunk.T @ P over k-chunks (PSUM-resident).
  - denom via ones-matmul; transposed to a per-partition column through a
    DRAM bounce; reciprocal on DVE; applied in the y epilogue.
  - y[q, n] = (outT.T @ WoT) * recip[q] + bo.

Matmul chains that accumulate into one PSUM bank serialize on the PE
(array drain between dependent matmuls), so independent chains are
emitted pairwise interleaved throughout.
"""

import os
import numpy as np

import concourse.bass as bass
import concourse.bacc as bacc
import concourse.mybir as mybir
import concourse.tile as tile
from concourse.bass_utils import run_bass_kernel_spmd

P = 128
B, S, D = 4, 2048, 1024
NCORES = 8
QL = S * B // NCORES          # 1024 queries per core
DT = mybir.dt

LAST_EXEC_TIME_NS = None
_CACHE = {}


def _build():
    nc = bacc.Bacc("TRN2", target_bir_lowering=False, debug=False,
                   enable_asserts=True, num_devices=NCORES)

    x_in = nc.dram_tensor("x", [S, D], DT.float32, kind="ExternalInput")
    w_in = {w: nc.dram_tensor(w, [D, D], DT.float32, kind="ExternalInput")
            for w in ("Wq", "Wk", "Wv", "Wo")}
    mask_in = nc.dram_tensor("mask2d", [P, S // P], DT.float32,
                             kind="ExternalInput")
    bq_in = nc.dram_tensor("bq2d", [P, D // P], DT.float32, kind="ExternalInput")
    bk_in = nc.dram_tensor("bk2d", [P, D // P], DT.float32, kind="ExternalInput")
    bv_in = nc.dram_tensor("bv2d", [1, D], DT.float32, kind="ExternalInput")
    bo_in = nc.dram_tensor("bo2d", [1, D], DT.float32, kind="ExternalInput")
    y_out = nc.dram_tensor("y", [QL, D], DT.float32, kind="ExternalOutput")

    DC = D // P       # 8 d-chunks
    KC = S // P       # 16 k-chunks
    QCH = 256         # query chunk (psum-bank limited)
    NQC = QL // QCH   # 4 query chunks

    with tile.TileContext(nc) as tc:
        with (
            tc.tile_pool(name="const", bufs=1) as cpool,
            tc.tile_pool(name="big", bufs=1) as big,
            tc.tile_pool(name="wpool", bufs=2) as wpool,
            tc.tile_pool(name="ppool", bufs=4) as ppool,
            tc.tile_pool(name="otpool", bufs=2) as otpool,
            tc.tile_pool(name="ypool", bufs=3) as ypool,
            tc.tile_pool(name="small", bufs=2) as small,
            tc.tile_pool(name="psmm", bufs=3, space="PSUM") as psmm,
            tc.tile_pool(name="psout", bufs=4, space="PSUM") as psout,
            tc.tile_pool(name="psden", bufs=1, space="PSUM") as psden,
            tc.tile_pool(name="dram", bufs=1, space="DRAM") as dram,
        ):
            # ---- constants ----
            mask_sb = cpool.tile([P, KC], DT.float32, tag="mask")
            nc.sync.dma_start(mask_sb[:], mask_in[:])
            bq_sb = cpool.tile([P, DC], DT.float32, tag="bq")
            nc.sync.dma_start(bq_sb[:], bq_in[:])
            bk_sb = cpool.tile([P, DC], DT.float32, tag="bk")
            nc.sync.dma_start(bk_sb[:], bk_in[:])
            bv_bc = cpool.tile([P, D], DT.float32, tag="bv")
            nc.sync.dma_start(bv_bc[:], bv_in[:].to_broadcast((P, D)))
            bo_bc = cpool.tile([P, D], DT.float32, tag="bo")
            nc.sync.dma_start(bo_bc[:], bo_in[:].to_broadcast((P, D)))
            ones = cpool.tile([P, 1], DT.bfloat16, tag="ones")
            nc.vector.memset(ones[:], 1.0)

            # ---- x -> bf16 (SWDGE casting DMA) -> DMA-transpose to xT ----
            xbf = dram.tile([S, D], DT.bfloat16, tag="xbf")
            for mb in range(4):
                nc.gpsimd.dma_start(xbf[mb * 512:(mb + 1) * 512, :],
                                    x_in[mb * 512:(mb + 1) * 512, :])
            xt = big.tile([P, DC, S], DT.bfloat16, tag="xt")
            for dj in range(DC):
                for mb in range(4):
                    nc.sync.dma_start_transpose(
                        xt[:, dj, mb * 512:(mb + 1) * 512],
                        xbf[mb * 512:(mb + 1) * 512, dj * P:(dj + 1) * P],
                    )

            def prep_weight(name):
                wbf = dram.tile([D, D], DT.bfloat16, tag="wbf")
                for mb in range(2):
                    nc.gpsimd.dma_start(wbf[mb * 512:(mb + 1) * 512, :],
                                        w_in[name][mb * 512:(mb + 1) * 512, :])
                wt = wpool.tile([P, DC, D], DT.bfloat16, tag="wT")
                for dj in range(DC):
                    for mb in range(2):
                        nc.sync.dma_start_transpose(
                            wt[:, dj, mb * 512:(mb + 1) * 512],
                            wbf[mb * 512:(mb + 1) * 512, dj * P:(dj + 1) * P],
                        )
                return wt

            def mm_chain_pair(specs):
                """specs: list of (psum_ap, lhsT_fn, rhs_fn) emitted with the
                DC-long accumulation chains interleaved so the PE array drain
                of one chain overlaps the stream of the other."""
                for dc in range(DC):
                    for ps, lhsT_fn, rhs_fn in specs:
                        nc.tensor.matmul(ps, lhsT_fn(dc), rhs_fn(dc),
                                         start=(dc == 0), stop=(dc == DC - 1))

            # ---- Q.T / K.T projections: [df, m] ----
            def proj_T(wt, bias_sb, out_t, m_size):
                tiles = [(nq, mh) for nq in range(DC)
                         for mh in range(m_size // 512)]
                for i in range(0, len(tiles), 2):
                    pair = tiles[i:i + 2]
                    pss = []
                    for nq, mh in pair:
                        ps = psmm.tile([P, 512], DT.float32, tag="mm",
                                       name=f"ps_{out_t.name}_{nq}_{mh}")
                        pss.append(ps)
                    mm_chain_pair([
                        (ps[:],
                         (lambda dc, nq=nq: wt[:, dc, nq * P:(nq + 1) * P]),
                         (lambda dc, mh=mh: xt[:, dc, mh * 512:(mh + 1) * 512]))
                        for ps, (nq, mh) in zip(pss, pair)])
                    for ps, (nq, mh) in zip(pss, pair):
                        nc.vector.tensor_scalar_add(
                            out_t[:, nq, mh * 512:(mh + 1) * 512], ps[:],
                            bias_sb[:, nq:nq + 1])

            wqt = prep_weight("Wq")
            qt = big.tile([P, DC, QL], DT.bfloat16, tag="qt")
            proj_T(wqt, bq_sb, qt, QL)

            wkt = prep_weight("Wk")
            kt = big.tile([P, DC, S], DT.bfloat16, tag="kt")
            proj_T(wkt, bk_sb, kt, S)

            # ---- V projection (natural layout): [k, dv] ----
            wvt = prep_weight("Wv")
            v = big.tile([P, KC, D], DT.bfloat16, tag="v")
            vtiles = [(kc, dh) for kc in range(KC) for dh in range(D // 512)]
            for i in range(0, len(vtiles), 2):
                pair = vtiles[i:i + 2]
                pss = [psmm.tile([P, 512], DT.float32, tag="mm",
                                 name=f"ps_v_{kc}_{dh}") for kc, dh in pair]
                mm_chain_pair([
                    (ps[:],
                     (lambda dc, kc=kc: xt[:, dc, kc * P:(kc + 1) * P]),
                     (lambda dc, dh=dh: wvt[:, dc, dh * 512:(dh + 1) * 512]))
                    for ps, (kc, dh) in zip(pss, pair)])
                for ps, (kc, dh) in zip(pss, pair):
                    nc.vector.tensor_tensor(
                        v[:, kc, dh * 512:(dh + 1) * 512], ps[:],
                        bv_bc[:, dh * 512:(dh + 1) * 512],
                        mybir.AluOpType.add)

            wot = prep_weight("Wo")

            # ---- attention + output projection, per query chunk ----
            for qc in range(NQC):
                q0 = qc * QCH
                out_ps = [psout.tile([P, 2, QCH], DT.float32, tag="outps",
                                     name=f"outps_{qc}_{j}")
                          for j in range(4)]
                den_ps = psden.tile([1, QCH], DT.float32, tag="den")

                def scores_pair(j):
                    kcs = [2 * j, 2 * j + 1]
                    pss = [psmm.tile([P, QCH], DT.float32, tag="mm",
                                     name=f"s_ps_{qc}_{kc}") for kc in kcs]
                    mm_chain_pair([
                        (ps[:],
                         (lambda dc, kc=kc: kt[:, dc, kc * P:(kc + 1) * P]),
                         (lambda dc: qt[:, dc, q0:q0 + QCH]))
                        for ps, kc in zip(pss, kcs)])
                    pts = []
                    for ps, kc in zip(pss, kcs):
                        p_t = ppool.tile([P, QCH], DT.bfloat16, tag="p",
                                         name=f"p_{qc}_{kc}")
                        nc.scalar.activation(p_t[:], ps[:],
                                             mybir.ActivationFunctionType.Exp,
                                             scale=mask_sb[:, kc:kc + 1])
                        pts.append(p_t)
                    return pts

                pair_prev = scores_pair(0)
                for j in range(KC // 2):
                    pair_next = scores_pair(j + 1) if j + 1 < KC // 2 else None
                    for kc, p_t in zip((2 * j, 2 * j + 1), pair_prev):
                        first, last = (kc == 0), (kc == KC - 1)
                        for dvc in range(DC):
                            # start=True clears has_written for the WHOLE
                            # bank: only the first write of a bank-sharing
                            # pair may set it.
                            nc.tensor.matmul(
                                out_ps[dvc // 2][:, dvc % 2, :],
                                v[:, kc, dvc * P:(dvc + 1) * P], p_t[:],
                                start=(first and dvc % 2 == 0), stop=last)
                        nc.tensor.matmul(den_ps[:], ones[:], p_t[:],
                                         start=first, stop=last)
                    pair_prev = pair_next

                # denominator -> per-partition reciprocal column [128, 2]
                dsb = small.tile([1, QCH], DT.float32, tag="dsb")
                nc.vector.tensor_copy(dsb[:], den_ps[:])
                dtmp = dram.tile([QCH], DT.float32, tag="dtmp",
                                 name=f"dtmp_{qc}")
                nc.sync.dma_start(dtmp.rearrange("(a q) -> a q", a=1), dsb[:])
                dcol = small.tile([P, QCH // P], DT.float32, tag="dcol")
                nc.sync.dma_start(dcol[:],
                                  dtmp.rearrange("(j p) -> p j", p=P))
                rcol = small.tile([P, QCH // P], DT.float32, tag="rcol")
                nc.vector.reciprocal(rcol[:], dcol[:])

                # outT psum -> sbuf bf16 (unnormalized)
                ot = otpool.tile([P, DC, QCH], DT.bfloat16, tag="ot")
                for j in range(4):
                    nc.vector.tensor_copy(ot[:, 2 * j:2 * j + 2, :],
                                          out_ps[j][:])

                # y[q, n] = (ot.T @ WoT) * recip[q] + bo
                ytiles = [(qs, nh) for qs in range(QCH // P)
                          for nh in range(D // 512)]
                for i in range(0, len(ytiles), 2):
                    pair = ytiles[i:i + 2]
                    pss = [psmm.tile([P, 512], DT.float32, tag="mm",
                                     name=f"y_ps_{qc}_{qs}_{nh}")
                           for qs, nh in pair]
                    mm_chain_pair([
                        (ps[:],
                         (lambda dvc, qs=qs: ot[:, dvc, qs * P:(qs + 1) * P]),
                         (lambda dvc, nh=nh: wot[:, dvc,
                                                 nh * 512:(nh + 1) * 512]))
                        for ps, (qs, nh) in zip(pss, pair)])
                    for ps, (qs, nh) in zip(pss, pair):
                        ysb = ypool.tile([P, 512], DT.float32, tag="y")
                        nc.vector.tensor_scalar_mul(ysb[:], ps[:],
                                                    rcol[:, qs:qs + 1])
                        nc.vector.tensor_tensor(
                            ysb[:], ysb[:], bo_bc[:, nh * 512:(nh + 1) * 512],
                            mybir.AluOpType.add)
                        nc.sync.dma_start(
                            y_out[q0 + qs * P:q0 + (qs + 1) * P,
                                  nh * 512:(nh + 1) * 512], ysb[:])

    nc.compile()
    return nc


def _host_inputs(x, Wq, bq, Wk, bk, Wv, bv, Wo, bo):
    pos = np.arange(S, dtype=np.float32)
    gauss = np.exp((-0.5 * ((pos - S / 2) / (S / 4)) ** 2).astype(np.float32))
    scale_vec = (gauss / np.float32(np.sqrt(np.float32(D)))).astype(np.float32)

    common = {
        "Wq": np.ascontiguousarray(Wq, np.float32),
        "Wk": np.ascontiguousarray(Wk, np.float32),
        "Wv": np.ascontiguousarray(Wv, np.float32),
        "Wo": np.ascontiguousarray(Wo, np.float32),
        "bq2d": np.ascontiguousarray(np.asarray(bq, np.float32)
                                     .reshape(D // P, P).T),
        "bk2d": np.ascontiguousarray(np.asarray(bk, np.float32)
                                     .reshape(D // P, P).T),
        "bv2d": np.ascontiguousarray(np.asarray(bv, np.float32)
                                     .reshape(1, D)),
        "bo2d": np.ascontiguousarray(np.asarray(bo, np.float32)
                                     .reshape(1, D)),
    }
    in_maps = []
    for c in range(NCORES):
        b, h = divmod(c, 2)
        q0 = h * QL
        x_roll = np.roll(np.asarray(x[b], np.float32), -q0, axis=0)
        m_roll = np.roll(scale_vec, -q0)
        in_maps.append(dict(
            common,
            x=np.ascontiguousarray(x_roll),
            mask2d=np.ascontiguousarray(m_roll.reshape(S // P, P).T),
        ))
    return in_maps


def kernel(x, Wq, bq, Wk, bk, Wv, bv, Wo, bo):
    global LAST_EXEC_TIME_NS
    x = np.asarray(x, np.float32)
    if "nc" not in _CACHE:
        _CACHE["nc"] = _build()
    nc = _CACHE["nc"]
    in_maps = _host_inputs(x, Wq, bq, Wk, bk, Wv, bv, Wo, bo)
    trace = bool(int(os.environ.get("BASS_KERNEL_TRACE", "0")))
    res = run_bass_kernel_spmd(nc, in_maps, core_ids=list(range(NCORES)),
                               trace=trace)
    LAST_EXEC_TIME_NS = res.exec_time_ns
    y = np.empty((B, S, D), np.float32)
    for c in range(NCORES):
        b, h = divmod(c, 2)
        y[b, h * QL:(h + 1) * QL] = res.results[c]["y"]
    return y
